# revision 40
# baseline (speedup 1.0000x reference)
"""Causal single-head attention (B=4, S=2048, E=1024) on 8 TRN2 NeuronCores.

Key-split sharding: core (b, par) handles batch b = core//2 and OWNS the
alternating 256-wide key chunks of parity par (par=0: global chunks
0,2,4,6; par=1: 1,3,5,7).  Each core:

  - projects K^T and V (bf16) for only its 1024 owned keys,
  - projects Q^T (bf16) for all 2048 queries,
  - computes causally-trimmed scores per 128-query tile j against its
    owned chunks: a fixed SPMD shape of kext_j = j//4 + 1 key-chunk
    slots (40 slots total; on par=1 eight slots are causally dead and
    killed by an all -1e9 mask, data not program),
  - exp (no max-subtraction, scores are bounded); scores are computed
    TRANSPOSED [k, q] over query-tile PAIRS (which share a causal extent),
    so P feeds P^T @ V directly as lhsT — no PE transposes — and the
    denominator comes from an ap=1 matmul against a ones column,
  - outputs the UNNORMALIZED numerator [2048, 1024] bf16 and the partial
    softmax denominator as a [128, 16] f32 tile (query j*128+p at [p, j]).

The host combines the two half-key cores of each batch:
  out[b] = (num0 + num1) / (den0 + den1).

This removes the duplicated full-sequence K/V projections of a pure
query-split (each pair member projected K,V for all 2048 keys) at the
cost of duplicating the cheaper Q projection: ~440K PE cycles/core vs
~540K.  All matmul inputs are bf16 (PSUM accumulates f32); measured
end-to-end rel err ~6e-3 vs the f32 reference (gate 2e-2).

Schedule notes (measured on HW, ~211us max-core vs 312us for the
query-split f32r baseline; note the device has transient ~20% throttle
episodes — compare kernels within one window):
  - one shared 8-bank [P,512] PSUM pool for K/Q/V projections (pool
    transitions become per-bank pipelined deps, not barriers);
  - K proj runs dt-pair passes with et outermost so the first matmuls
    start as soon as the first (xk, wk) et-chunks land (~2us of data);
  - the 40 attention (j, s) slots are software-pipelined: slot s+1's
    score matmuls are emitted before slot s's exp/transpose/PV so the
    PE stays busy while the scalar engine runs exp;
  - input DMAs: xk/wk as 16 parallel et-chunks split across both HWDGE
    queues (sync + scalar); wq/xq/wv/masks as single big transfers
    coarsely chained behind them (each chain link costs ~3us trigger
    latency, so few links, and parallel chunks within a tensor);
  - output DMAs alternate the two queues; q-tiles run ascending so the
    output queue never backs up at the end (the last tile's write is
    split across both queues).
"""

import numpy as np
import ml_dtypes

B, S, E = 4, 2048, 1024
P = 128          # partitions
CH = 256         # key chunk
NCH = S // CH    # 8 global chunks, 4 owned per core
NEG = -1e9
NCORES = 8
SCALE = 1.0 / np.sqrt(np.float32(E))
ET = E // P      # 8 contraction tiles
DT = E // P      # 8 head-dim tiles
QT = S // P      # 16 query tiles

_CACHE = {}


def _install_drain_patch():
    """walrus in this env fits only 1 sync wait per CTRL_NO instruction; split
    the TileContext end-of-kernel drain waits across trailing SP nops."""
    import concourse.mybir as mybir
    import concourse.tile as tile
    from concourse.vector_clock import ScopedClock

    if getattr(tile.TileContext, "_drain_split_installed", False):
        return

    def _split_drain_and_barrier(self, tick_clock, wait_clock):
        drain_inst = self.nc.sync.drain()
        wait_clock.add_sem_waits(
            drain_inst.ins, ScopedClock({None: tick_clock.global_clock})
        )
        si = drain_inst.ins.sync_info
        waits = list(si.on_wait) if si and si.on_wait else []
        if len(waits) > 1:
            si.on_wait = waits[:1]
            rest = waits[1:]
            while rest:
                chunk, rest = rest[:1], rest[1:]
                nop = self.nc.sync.nop(nofuse=True, hint="drain_wait_split")
                nsi = nop.ins.sync_info
                if nsi is None:
                    nop.ins.sync_info = mybir.SyncInfo(on_wait=chunk, on_update=[])
                else:
                    nsi.on_wait = list(nsi.on_wait) + chunk

        self.nc.all_engine_barrier()
        assert self.sems is not None
        popped = self.nc._tile_sem_poison_stack.pop()
        assert popped is self._sem_poison
        # Skip the emitted clear_and_free_semaphores + second barrier (~5us):
        # NRT re-initializes semaphore state at NEFF launch, so end-of-kernel
        # cleanup is only needed when multiple TileContexts share one NEFF
        # (not the case here).  Verified by back-to-back re-executions of the
        # same loaded NEFF.  Keep the host-side bookkeeping only.
        sem_nums = [s.num for s in self.sems.allocated().values()]
        self.nc._state.prepend_free_semaphores(sem_nums)
        for poison_set in self.nc._tile_sem_poison_stack:
            poison_set.update(sem_nums)

    tile.TileContext._drain_and_barrier = _split_drain_and_barrier
    tile.TileContext._drain_split_installed = True


def _split_excess_waits(nc, limit=1):
    """walrus here fits only `limit` sync waits per instruction; move excess
    waits of every instruction onto injected same-engine NoOps placed directly
    before it (program order on the engine preserves the semantics)."""
    import copy

    import concourse.mybir as mybir

    template = None
    for f in nc.m.functions:
        for bb in f.blocks:
            for inst in bb.instructions:
                if type(inst).__name__ == "InstNoOp":
                    template = inst
                    break
            if template is not None:
                break
        if template is not None:
            break
    assert template is not None, "no InstNoOp template found"

    n = 0
    for f in nc.m.functions:
        for bb in f.blocks:
            new = []
            for inst in bb.instructions:
                si = inst.sync_info
                waits = list(si.on_wait) if si and si.on_wait else []
                if len(waits) > limit:
                    si.on_wait = waits[-limit:]
                    excess = waits[:-limit]
                    while excess:
                        chunk, excess = excess[:limit], excess[limit:]
                        nop = copy.copy(template)
                        nop.name = f"I-wsplit-{n}"
                        n += 1
                        nop.engine = inst.engine
                        nop.sync_info = mybir.SyncInfo(on_wait=chunk, on_update=[])
                        import bass_rust

                        nop.set_nosync_dependencies(
                            bass_rust.InstructionNameOrderedSet()
                        )
                        nop.set_sync_dependencies(
                            bass_rust.InstructionNameOrderedSet()
                        )
                        new.append(nop)
                new.append(inst)
            bb.instructions[:] = new
    return n


def _build_program():
    """One SPMD program; per-core behaviour differs only through input data."""
    import concourse.bass as bass
    import concourse.mybir as mybir
    import concourse.tile as tile
    from concourse.masks import make_identity
    from concourse.tile import add_dep_helper

    _install_drain_patch()

    f32 = mybir.dt.float32
    bf = mybir.dt.bfloat16
    Act = mybir.ActivationFunctionType

    nc = bass.Bass(dynamic_dma_scratch_size=128)
    xqT = nc.declare_dram_parameter("xqT", [E, S], bf, isOutput=False)
    xkT = nc.declare_dram_parameter("xkT", [E, S // 2], bf, isOutput=False)
    wq = nc.declare_dram_parameter("wq", [E, E], bf, isOutput=False)
    wk = nc.declare_dram_parameter("wk", [E, E], bf, isOutput=False)
    wv = nc.declare_dram_parameter("wv", [E, E], bf, isOutput=False)
    masks = nc.declare_dram_parameter("masks", [P, QT * CH + 8], bf, isOutput=False)
    out_num = nc.declare_dram_parameter("out_num", [S, E], bf, isOutput=True)
    out_den = nc.declare_dram_parameter("out_den", [P, QT], f32, isOutput=True)

    xqT_r = xqT.rearrange("(et p) s -> p et s", p=P)    # [128, 8, 2048]
    xkT_r = xkT.rearrange("(et p) s -> p et s", p=P)    # [128, 8, 1024]
    wq_r = wq.rearrange("(et p) d -> p et d", p=P)      # [128, 8, 1024]
    wk_r = wk.rearrange("(et p) d -> p et d", p=P)
    wv_r = wv.rearrange("(et p) d -> p et d", p=P)

    SK = S // 2          # 1024 owned keys
    NCHO = SK // CH      # 4 owned chunks
    KSUB = SK // P       # 8 owned 128-key subtiles

    with tile.TileContext(nc) as tc:
        from contextlib import ExitStack

        with ExitStack() as ctx:
            mpool = ctx.enter_context(tc.tile_pool(name="mask", bufs=1))
            kpool = ctx.enter_context(tc.tile_pool(name="kt", bufs=1))
            vpool = ctx.enter_context(tc.tile_pool(name="v", bufs=1))
            qpool = ctx.enter_context(tc.tile_pool(name="qt", bufs=1))

            ident = mpool.tile([P, P], f32)
            ident_bf = mpool.tile([P, P], bf)
            masks_sb = mpool.tile([P, QT * CH + 8], bf)

            kt_sb = kpool.tile([P, DT, SK], bf, tag="kt")     # K^T [d, k_own]
            v_t = [
                vpool.tile([P, E], bf, tag=f"v{ks}", name=f"v{ks}")
                for ks in range(KSUB)
            ]                                                  # V [k_own, d]
            qt_sbs = [
                qpool.tile([P, DT, 4 * P], bf, tag=f"qt{qb}", name=f"qt{qb}")
                for qb in range(4)
            ]                                                  # Q^T [d, q]

            # ---- K/V projection over owned keys + Q projection (all queries).
            # Per-et DMA chunks as separate tiles so the first matmul starts
            # as soon as the first 512KB lands; later streams are chained so
            # they don't steal HBM bandwidth from the critical path.
            with ExitStack() as pctx:
                wkp = pctx.enter_context(tc.tile_pool(name="wk", bufs=1))
                wvp = pctx.enter_context(tc.tile_pool(name="wv", bufs=1))
                wqp = pctx.enter_context(tc.tile_pool(name="wq", bufs=1))
                xkp = pctx.enter_context(tc.tile_pool(name="xk", bufs=1))
                xqp = pctx.enter_context(tc.tile_pool(name="xq", bufs=1))

                wk_t = [wkp.tile([P, E], bf, tag=f"wk{et}", name=f"wk{et}") for et in range(ET)]
                xk_t = [xkp.tile([P, SK], bf, tag=f"xk{et}", name=f"xk{et}") for et in range(ET)]
                wv_sb = wvp.tile([P, ET, E], bf, tag="wv")
                wq_sb = wqp.tile([P, ET, E], bf, tag="wq")
                xq_sb = xqp.tile([P, ET, S], bf, tag="xq")
                wq_t = [wq_sb[:, et, :] for et in range(ET)]
                wv_t = [wv_sb[:, et, :] for et in range(ET)]
                xq_t = [xq_sb[:, et, :] for et in range(ET)]

                # DMA priority: (xk,wk) et-chunks in parallel (K proj needs
                # them first); then ONE big DMA each for wq, xq, wv, masks,
                # coarsely chained — each chain link costs ~3us of trigger
                # latency, so fewer links = earlier arrival.
                kv_d = []
                for et in range(ET):
                    eng = nc.sync if et % 2 == 0 else nc.scalar
                    kv_d.append(eng.dma_start(xk_t[et][:], xkT_r[:, et, :]))
                    kv_d.append(eng.dma_start(wk_t[et][:], wk_r[:, et, :]))
                # Parallel chunks within each tensor (single DMAs only reach
                # ~170GB/s), coarse chain between tensors (links cost ~3us).
                wq_d = []
                for h in range(2):
                    dmh = nc.sync.dma_start(
                        wq_sb[:, bass.ts(h, ET // 2), :],
                        wq_r[:, bass.ts(h, ET // 2), :],
                    )
                    add_dep_helper(dmh.ins, kv_d[-1].ins, reason="dma chain")
                    wq_d.append(dmh)
                xq_d = []
                for h in range(4):
                    dmh = nc.sync.dma_start(
                        xq_sb[:, bass.ts(h, ET // 4), :],
                        xqT_r[:, bass.ts(h, ET // 4), :],
                    )
                    for prev in wq_d:
                        add_dep_helper(dmh.ins, prev.ins, reason="dma chain")
                    xq_d.append(dmh)
                wv_d = []
                for h in range(2):
                    dmh = nc.sync.dma_start(
                        wv_sb[:, bass.ts(h, ET // 2), :],
                        wv_r[:, bass.ts(h, ET // 2), :],
                    )
                    add_dep_helper(dmh.ins, xq_d[-1].ins, reason="dma chain")
                    add_dep_helper(dmh.ins, xq_d[-2].ins, reason="dma chain")
                    wv_d.append(dmh)
                mask_d = nc.sync.dma_start(masks_sb[:], masks[:])
                add_dep_helper(mask_d.ins, wv_d[-1].ins, reason="dma chain")
                add_dep_helper(mask_d.ins, wv_d[-2].ins, reason="dma chain")

                # identity AFTER the DMA kicks: keeps the Sync queue free so
                # the first xk/wk triggers fire right after the entry barrier
                make_identity(nc, ident)
                nc.vector.tensor_copy(ident_bf[:], ident[:])

                # One shared PSUM pool for K/Q/V with 8 uniform [P,512] f32
                # banks (tags b0..b7): phase transitions become per-bank
                # pipelined dependencies instead of pool-close barriers.
                with ExitStack() as pjctx:
                    pjps = pjctx.enter_context(
                        tc.tile_pool(name="projpsum", bufs=1, space="PSUM")
                    )

                    def bank(i, name):
                        return pjps.tile([P, 4 * P], f32, tag=f"b{i}", name=name)

                    # K^T: [d, k_own].  dt-PAIR passes with et outermost: the
                    # first matmul starts as soon as (xk0, wk0) land, and
                    # pass 1 consumes et chunks at the DMA arrival rate.
                    # 8 psums per pass = 4 chunks x 2 dt; copies are emitted
                    # right after each psum's stop so later passes never stall.
                    for dp in range(DT // 2):
                        pps = {}
                        for ch in range(NCHO):
                            for dl in range(2):
                                pps[(ch, dl)] = bank(
                                    ch * 2 + dl, f"kp{dp}_{ch}_{dl}"
                                )
                        for et in range(ET):
                            for ch in range(NCHO):
                                for dl in range(2):
                                    dt = dp * 2 + dl
                                    nc.tensor.matmul(
                                        pps[(ch, dl)][:, 0:CH],
                                        wk_t[et][:, bass.ts(dt, P)],
                                        xk_t[et][:, bass.ts(ch, CH)],
                                        start=(et == 0),
                                        stop=(et == ET - 1),
                                    )
                                    if et == ET - 1:
                                        nc.vector.tensor_copy(
                                            kt_sb[:, dt, bass.ts(ch, CH)],
                                            pps[(ch, dl)][:, 0:CH],
                                        )

                    # Q^T: [d, q] for all queries, in 512-col blocks
                    for qb in range(4):
                        qps = [bank(dt, f"qp{qb}_{dt}") for dt in range(DT)]
                        for et in range(ET):
                            for dt in range(DT):
                                nc.tensor.matmul(
                                    qps[dt][:],
                                    wq_t[et][:, bass.ts(dt, P)],
                                    xq_t[et][:, bass.ts(qb, 4 * P)],
                                    start=(et == 0),
                                    stop=(et == ET - 1),
                                )
                                if et == ET - 1:
                                    nc.vector.tensor_copy(
                                        qt_sbs[qb][:, dt, :], qps[dt][:]
                                    )

                    # V: [k_own, d] per owned 128-subtile (x stationary lhsT)
                    for ks in range(KSUB):
                        for db in range(2):
                            pp = bank((ks * 2 + db) % 8, f"vp{ks}_{db}")
                            for et in range(ET):
                                nc.tensor.matmul(
                                    pp[:],
                                    xk_t[et][:, bass.ts(ks, P)],
                                    wv_t[et][:, bass.ts(db, E // 2)],
                                    start=(et == 0),
                                    stop=(et == ET - 1),
                                )
                            nc.vector.tensor_copy(
                                v_t[ks][:, bass.ts(db, E // 2)], pp[:]
                            )

            # ---- attention: per 128-query tile j, kext = j//4+1 owned-chunk
            # slots; last slot's mask data is triangle / zeros / all-dead.
            with ExitStack() as actx:
                ppool = actx.enter_context(tc.tile_pool(name="p", bufs=4))
                obuf = actx.enter_context(tc.tile_pool(name="ob", bufs=4))
                stat = actx.enter_context(tc.tile_pool(name="stat", bufs=8))
                spsum = actx.enter_context(
                    tc.tile_pool(name="spsum", bufs=2, space="PSUM")
                )
                opsum = actx.enter_context(
                    tc.tile_pool(name="opsum", bufs=1, space="PSUM")
                )

                # TRANSPOSED [k, q] scores over query-tile PAIRS (2p, 2p+1):
                # both share kext = p//2+1, so score blocks are [128k, 256q]
                # and the PE P^T transposes + PSUM->SBUF copies disappear;
                # PV uses P directly as lhsT; den via ap=1 ones-matmul.
                subslots = []
                for p in range(QT // 2):
                    kext = p // 2 + 1
                    for s in range(kext):
                        for k2 in range(2):
                            subslots.append((p, s, k2, s == kext - 1))

                def emit_scores(p, s, k2, is_last):
                    s_t = spsum.tile([P, CH], f32, tag="s", name=f"s{p}_{s}_{k2}")
                    qt_j = qt_sbs[p // 2]
                    qcols = bass.ds((p % 2) * 2 * P, 2 * P)
                    for dt in range(DT):
                        nc.tensor.matmul(
                            s_t[:],
                            kt_sb[:, dt, bass.ds(s * CH + k2 * P, P)],
                            qt_j[:, dt, qcols],
                            start=(dt == 0),
                            stop=(dt == DT - 1 and not is_last),
                        )
                    if is_last:
                        nc.tensor.matmul(
                            s_t[:],
                            ident_bf[:],
                            masks_sb[:, bass.ts(2 * p + k2, CH)],
                            start=False,
                            stop=True,
                        )
                    return s_t

                ones_bf = masks_sb[:, QT * CH : QT * CH + 1]
                den_all = stat.tile([P, QT], f32, tag="den_all")
                pstate = {}
                pend = emit_scores(*subslots[0])
                for idx, (p, s, k2, is_last) in enumerate(subslots):
                    s_t = pend
                    if idx + 1 < len(subslots):
                        pend = emit_scores(*subslots[idx + 1])
                    if p not in pstate:
                        pstate[p] = [
                            (
                                opsum.tile(
                                    [P, E // 2], f32, tag=f"olo{jh}",
                                    name=f"olo{p}_{jh}",
                                ),
                                opsum.tile(
                                    [P, E // 2], f32, tag=f"ohi{jh}",
                                    name=f"ohi{p}_{jh}",
                                ),
                                opsum.tile(
                                    [P, 1], f32, tag=f"od{jh}",
                                    name=f"od{p}_{jh}",
                                ),
                            )
                            for jh in range(2)
                        ]
                    p_t = ppool.tile([P, CH], bf, tag="p", name=f"p{p}_{s}_{k2}")
                    nc.scalar.activation(
                        p_t[:], s_t[:], Act.Exp, scale=float(SCALE)
                    )
                    ks = s * 2 + k2
                    first = s == 0 and k2 == 0
                    lastpv = is_last and k2 == 1
                    for jh in range(2):
                        o_lo, o_hi, o_den = pstate[p][jh]
                        lhs = p_t[:, bass.ts(jh, P)]
                        nc.tensor.matmul(
                            o_lo[:], lhs, v_t[ks][:, 0 : E // 2],
                            start=first, stop=lastpv,
                        )
                        nc.tensor.matmul(
                            o_hi[:], lhs, v_t[ks][:, E // 2 : E],
                            start=first, stop=lastpv,
                        )
                        nc.tensor.matmul(
                            o_den[:], lhs, ones_bf,
                            start=first, stop=lastpv,
                        )
                    if lastpv:
                        for jh in range(2):
                            j = 2 * p + jh
                            o_lo, o_hi, o_den = pstate[p][jh]
                            nc.vector.tensor_copy(
                                den_all[:, j : j + 1], o_den[:]
                            )
                            ob = obuf.tile([P, E], bf, tag="ob", name=f"ob{j}")
                            if j == QT - 1:
                                nc.vector.tensor_copy(ob[:, 0 : E // 2], o_lo[:])
                                nc.scalar.activation(
                                    ob[:, E // 2 : E], o_hi[:], Act.Copy
                                )
                            else:
                                nc.vector.tensor_copy(ob[:, 0 : E // 2], o_lo[:])
                                nc.vector.tensor_copy(ob[:, E // 2 : E], o_hi[:])
                            if j >= QT - 2:
                                nc.scalar.dma_start(
                                    out_num[bass.ds(j * P, P), 0 : E // 2],
                                    ob[:, 0 : E // 2],
                                )
                                nc.sync.dma_start(
                                    out_num[bass.ds(j * P, P), E // 2 : E],
                                    ob[:, E // 2 : E],
                                )
                            else:
                                eng = nc.scalar if j % 2 == 0 else nc.sync
                                eng.dma_start(
                                    out_num[bass.ds(j * P, P), :], ob[:]
                                )
                        del pstate[p]
                nc.sync.dma_start(out_den[:], den_all[:])
    _split_excess_waits(nc)
    return nc


def _own_cols(par):
    """Column indices (into the 2048 sequence) of the owned key chunks."""
    chunks = range(par, NCH, 2)
    return np.concatenate([np.arange(c * CH, (c + 1) * CH) for c in chunks])


def _build_masks(par):
    """Slice sp = 2*p + k2 ([128 k, 256 q], bf16) added to the scores of the
    LAST owned-chunk slot of query pair p, k-subtile k2: causal triangle if
    this core owns pair p's diagonal chunk (global chunk p), all -1e9 if the
    slot is causally dead (par=1, even p), zeros otherwise.  Column QT*CH
    holds the ones vector for the denominator matmul."""
    m = np.zeros((P, QT * CH + 8), np.float32)
    kp = np.arange(P)[:, None]
    q = np.arange(CH)[None, :]
    for p in range(QT // 2):
        for k2 in range(2):
            sl = m[:, (2 * p + k2) * CH : (2 * p + k2 + 1) * CH]
            if p % 2 == par:
                sl[:] = np.where(q >= k2 * P + kp, 0.0, np.float32(NEG))
            elif par == 1:
                sl[:] = NEG
    m[:, QT * CH] = 1.0
    return np.ascontiguousarray(m.astype(ml_dtypes.bfloat16))


def _bf16(a):
    return np.ascontiguousarray(a.astype(ml_dtypes.bfloat16))


def build_in_maps(x, W_Q, W_K, W_V):
    x = np.asarray(x, dtype=np.float32)
    wqT = _bf16(np.asarray(W_Q, np.float32).T)
    wkT = _bf16(np.asarray(W_K, np.float32).T)
    wvT = _bf16(np.asarray(W_V, np.float32).T)
    in_maps = []
    for c in range(NCORES):
        b, par = c // 2, c % 2
        xT = x[b].T  # [E, S]
        in_maps.append(
            {
                "xqT": _bf16(xT),
                "xkT": _bf16(xT[:, _own_cols(par)]),
                "wq": wqT,
                "wk": wkT,
                "wv": wvT,
                "masks": _build_masks(par),
            }
        )
    return in_maps


def combine(results):
    out = np.empty((B, S, E), np.float32)
    for b in range(B):
        r0, r1 = results[2 * b], results[2 * b + 1]
        num = r0["out_num"].astype(np.float32) + r1["out_num"].astype(np.float32)
        den = (r0["out_den"] + r1["out_den"]).T.reshape(S, 1)
        out[b] = num / den
    return out


def kernel(x, W_Q, W_K, W_V):
    from concourse.bass_utils import run_bass_kernel_spmd

    if "nc" not in _CACHE:
        _CACHE["nc"] = _build_program()
    nc = _CACHE["nc"]

    in_maps = build_in_maps(x, W_Q, W_K, W_V)
    res = run_bass_kernel_spmd(nc, in_maps, list(range(NCORES)))
    return combine(res.results)
